# revision 41
# baseline (speedup 1.0000x reference)
"""Multi-head attention (B=4, S=2048, D=512, H=8) on 8 Trainium2 NeuronCores.

Sharding: core c handles batch b = c//2 and heads [4*(c%2) .. 4*(c%2)+3]
(data parallel on B, tensor parallel on H). Each core computes Q/K/V
projections for its 4 heads, per-head attention, and a partial output
projection (its 256 rows of Wo). The host sums the two partial outputs per
batch and adds bo.

Perf structure (fp32 baseline 522us -> fp16 151us -> this, ~133us):
 - All matmul operands fp16: 1 PE cycle/column (fp32 is 4). ~1e-3 rel err.
 - Masked keys are compacted away on the host (padded to a multiple of 128);
   padding keys produce score 0 -> exp 1, but their V' rows and ones-column
   are 0 so they contribute nothing to AV numerator or softmax denominator.
 - The Scalar engine's exp is the attention pacer (~1.1us per [128,1024]
   instruction covering both heads of a pair, 72 instructions, back-to-back
   for ~87us). Everything else is arranged to hide under or around it:
     * all input DMAs share one queue in priority order (K inputs first, Q
       chunks last) — the 16 underlying DMA engines are bandwidth-shared
       (~350GB/s aggregate), so competing queues would starve the first
       projections; junk matmuls warm the PE clock (0.65->2.4GHz) and a
       dummy exp preloads the activation table while inputs stream;
     * only K0/V/Q0-chunk0 projections run up front; remaining Q0 chunks
       and pair-1's K/Q chains are interleaved into pair-0's attention,
       borrowing score-ring PSUM banks in the PE's idle gaps;
     * softmax denominators are folded into V' (65th row); each query chunk
       is normalized eagerly right after its AV chain using
       reciprocal_approx_fast (single DVE op) and one DRAM round trip to
       broadcast 1/den across partitions (the odd head's O rows bounce
       through DRAM anyway to reach partitions 64..127);
     * the output projection reuses the score ring's PSUM tiles inside the
       attention pools' scope (a fresh pool would pay a 15-20us transition
       barrier); PSUM evacuation casts alternate Vector/Scalar (the only
       PSUM-capable engines) and 4 grouped DMAs ship the fp16 result.
 - Empirical pitfalls baked into the structure: SBUF layout is
   perf-sensitive (shifting the exp output tile by 128B cost 220ns on every
   exp — keep late-added tiles allocated after the hot ones); mixing
   K=128 and K=64 matmuls in one PSUM accumulation group hard-faults the
   device; GPSIMD and DMA cannot touch PSUM.
"""

import numpy as np
from contextlib import ExitStack

import concourse.bass as bass
from concourse.bacc import Bacc
import concourse.mybir as mybir
import concourse.tile as tile
from concourse import bass_utils

F32 = mybir.dt.float32
F16 = mybir.dt.float16
B, S, D, H, HD = 4, 2048, 512, 8, 64
P = 128
HPC = 4            # heads per core
NSQ = S // 512     # 4 query chunks of 512


def _build(aug: bool, nsk: int) -> bass.Bass:
    kt = 5 if aug else 4
    da = kt * P
    SK = nsk * P
    nc = Bacc(trn_type="TRN2")

    xT = nc.dram_tensor("xT", [da, S], F16, kind="ExternalInput")
    xkT = nc.dram_tensor("xkT", [da, SK], F16, kind="ExternalInput")
    wk = nc.dram_tensor("wk", [da, HPC * HD], F16, kind="ExternalInput")
    wv = nc.dram_tensor("wv", [da, HPC * 65], F16, kind="ExternalInput")
    wq = nc.dram_tensor("wq", [da, HPC * HD], F16, kind="ExternalInput")
    wo = nc.dram_tensor("wo", [2, P, D], F16, kind="ExternalInput")
    maskt = nc.dram_tensor("maskt", [P, nsk], F32, kind="ExternalInput")
    out = nc.dram_tensor("out", [S, D], F16, kind="ExternalOutput")

    with tile.TileContext(nc) as tc, ExitStack() as ctx:
        sb = ctx.enter_context(tc.tile_pool(name="sb", bufs=1))
        dram = ctx.enter_context(tc.tile_pool(name="dram", bufs=1, space="DRAM"))

        # ---------- input DMAs: one queue, priority order ----------
        xTr = xT.rearrange("(t p) m -> p t m", p=P)
        wkt = sb.tile([P, kt, HPC * HD], F16)
        nc.sync.dma_start(wkt[:], wk.rearrange("(t p) m -> p t m", p=P))
        xkt = sb.tile([P, kt, SK], F16)
        xkr = xkT.rearrange("(t p) m -> p t m", p=P)
        for (lo, hi) in [(c, min(c + 512, SK)) for c in range(0, SK, 512)]:
            nc.sync.dma_start(xkt[:, :, lo:hi], xkr[:, :, lo:hi])
        mkt = sb.tile([P, nsk], F32)
        nc.sync.dma_start(mkt[:], maskt[:])
        wvt = sb.tile([P, kt, HPC * 65], F16)
        nc.sync.dma_start(wvt[:], wv.rearrange("(t p) m -> p t m", p=P))
        wqt = sb.tile([P, kt, HPC * HD], F16)
        nc.sync.dma_start(wqt[:], wq.rearrange("(t p) m -> p t m", p=P))
        xt = sb.tile([P, kt, S], F16)
        for j in range(NSQ):
            nc.sync.dma_start(xt[:, :, j * 512:(j + 1) * 512],
                              xTr[:, :, j * 512:(j + 1) * 512])
        wot = sb.tile([P, 2, D], F16)
        nc.sync.dma_start(wot[:], wo.rearrange("m p d -> p m d"))

        ktile = [sb.tile([P, SK], F16, tag=f"kT{m}", name=f"kT{m}") for m in range(2)]
        qtile = [sb.tile([P, S], F16, tag=f"qT{m}", name=f"qT{m}") for m in range(2)]
        vt = sb.tile([P, nsk, HPC * 65], F16)
        # one O' tile per (pair, query chunk): the output projection's
        # dependency tracking is per-tile, so chunk-sized tiles let early
        # column chunks project while the last chunk still normalizes
        opj = [[sb.tile([P, 512], F16, tag=f"op{m}_{j}", name=f"op{m}_{j}")
                for j in range(NSQ)] for m in range(2)]
        oscr = dram.tile([2, NSQ, HD, 512], F16)           # odd-head O bounce
        rscr = dram.tile([2, NSQ, 2, 512], F16)            # 1/den bounce
        kchunks = [(c, min(c + 512, SK)) for c in range(0, SK, 512)]

        def cast(eng, dst, src):
            if eng is nc.scalar:
                nc.scalar.copy(dst, src)
            else:
                eng.tensor_copy(dst, src)

        def proj_k(m, pool, lo, hi, eng=nc.vector):
            ps = pool.tile([P, 1024], F32, tag="sc")
            for t in range(kt):
                nc.tensor.matmul(
                    ps[:, 0:hi - lo], wkt[:, t, m * P:(m + 1) * P],
                    xkt[:, t, lo:hi], start=(t == 0), stop=(t == kt - 1))
            cast(eng, ktile[m][:, lo:hi], ps[:, 0:hi - lo])

        def proj_v(pool, si, eng=nc.vector):
            # natural orientation [keys, 4*65], all four heads at once, two
            # key chunks per PSUM tile; the ones-columns then get 1*mask
            # (key-validity) via tiny gpsimd ops
            n2 = min(2, nsk - si)
            ps = pool.tile([P, 1024], F32, tag="sc")
            for h in range(n2):
                for t in range(kt):
                    nc.tensor.matmul(
                        ps[:, h * 512:h * 512 + HPC * 65],
                        xkt[:, t, (si + h) * P:(si + h + 1) * P],
                        wvt[:, t, :], start=(t == 0), stop=(t == kt - 1))
            for h in range(n2):
                cast(eng, vt[:, si + h, :], ps[:, h * 512:h * 512 + HPC * 65])
            if not aug:
                for h in range(n2):
                    ones = vt[:, si + h, HD::65]
                    nc.gpsimd.memset(ones, 1.0)
                    nc.gpsimd.tensor_scalar_mul(ones, ones, mkt[:, si + h:si + h + 1])

        def proj_q(m, pool, j, eng=nc.vector):
            ps = pool.tile([P, 1024], F32, tag="sc")
            for t in range(kt):
                nc.tensor.matmul(
                    ps[:, 0:512], wqt[:, t, m * P:(m + 1) * P],
                    xt[:, t, j * 512:(j + 1) * 512],
                    start=(t == 0), stop=(t == kt - 1))
            cast(eng, qtile[m][:, j * 512:(j + 1) * 512], ps[:, 0:512])

        # ---------- phase 1: minimal up-front projections ----------
        with tc.tile_pool(name="proj_ps", bufs=2, space="PSUM") as ppool:
            # Junk matmul chains while the input DMAs stream in: keeps the
            # PE busy from the preamble on so its clock is fully ramped
            # (0.65 -> 2.4 GHz takes a few us of activity) when the real
            # projections start. Results are never read.
            junk = sb.tile([P, 512], F16, tag="junk")
            nc.vector.memset(junk[:], 0.0)
            # preload the Exp activation table while DMAs stream
            jp = sb.tile([P, 1], F16, tag="jp")
            nc.scalar.activation(jp[:], junk[:, 0:1],
                                 mybir.ActivationFunctionType.Exp)
            for _ in range(5):
                ps = ppool.tile([P, 1024], F32, tag="sc")
                for r in range(4):
                    nc.tensor.matmul(ps[:, 0:512], junk[:, 0:P], junk[:],
                                     start=(r == 0), stop=(r == 3))
            engs = [nc.vector, nc.scalar]
            n = 0
            for (lo, hi) in kchunks:
                proj_k(0, ppool, lo, hi, engs[n % 2]); n += 1
            for si in range(0, nsk, 2):
                proj_v(ppool, si, engs[n % 2]); n += 1
            proj_q(0, ppool, 0, engs[n % 2]); n += 1

        # chains to interleave into pair-0's attention (PE idle gaps)
        pending = []
        for (lo, hi) in kchunks:
            pending.append(lambda pool, lo=lo, hi=hi: proj_k(1, pool, lo, hi))
        for j in range(NSQ):
            pending.append(lambda pool, j=j: proj_q(1, pool, j))

        # ---------- phase 2: attention ----------
        with tc.tile_pool(name="attn_ps", bufs=2, space="PSUM") as apool, \
             tc.tile_pool(name="av_ps", bufs=2, space="PSUM") as avpool:
            for m in range(2):                              # head pair
                le, lo_ = 2 * m, 2 * m + 1
                if m == 1:
                    while pending:                          # safety net
                        pending.pop(0)(apool)
                for j in range(NSQ):                        # query chunk of 512
                    qe = qtile[m][0:HD, j * 512:(j + 1) * 512]
                    qo = qtile[m][HD:P, j * 512:(j + 1) * 512]
                    av_e = avpool.tile([65, 512], F32, tag="av_e")
                    av_o = avpool.tile([65, 512], F32, tag="av_o")

                    def emit_av(sk, p):
                        st = dict(start=(sk == 0), stop=(sk == nsk - 1))
                        nc.tensor.matmul(av_e[:], vt[:, sk, le * 65:le * 65 + 65],
                                         p[:, 0:512], **st)
                        nc.tensor.matmul(av_o[:], vt[:, sk, lo_ * 65:lo_ * 65 + 65],
                                         p[:, 512:1024], **st)

                    prev = None
                    for sk in range(nsk):
                        sc = apool.tile([P, 1024], F32, tag="sc")
                        nc.tensor.matmul(
                            sc[:, 0:512], ktile[m][0:HD, sk * P:(sk + 1) * P], qe,
                            start=True, stop=True)
                        nc.tensor.matmul(
                            sc[:, 512:1024], ktile[m][HD:P, sk * P:(sk + 1) * P], qo,
                            start=True, stop=True)
                        p = sb.tile([P, 1024], F16, tag="p", bufs=4)
                        nc.scalar.activation(p[:], sc[:],
                                             mybir.ActivationFunctionType.Exp,
                                             scale=0.125)
                        if prev is not None:
                            emit_av(*prev)
                        prev = (sk, p)
                        if m == 0:
                            # next Q0 chunk first, then pair-1 chains; late
                            # slots so the previous chunk's DVE burst (den
                            # chain + O copies) has drained and the inserted
                            # cast isn't queued behind it
                            if sk == 2 and j < NSQ - 1:
                                proj_q(0, apool, j + 1)
                            elif sk in (4, 6) and pending:
                                pending.pop(0)(apool)
                    emit_av(*prev)

                    # O rows out first so the odd head's DRAM bounce is in
                    # flight while the denominator chain runs
                    osh = sb.tile([HD, 512], F16, tag="osh", bufs=2)
                    nc.vector.tensor_copy(osh[:], av_o[0:HD, :])
                    nc.gpsimd.dma_start(oscr[m, j], osh[:])
                    nc.gpsimd.dma_start(opj[m][j][HD:P, :], oscr[m, j])
                    nc.vector.tensor_copy(opj[m][j][0:HD, :], av_e[0:HD, :])
                    # eager normalization: den rows -> 1/den (fast approx),
                    # broadcast across partitions by a DRAM round trip — or,
                    # for the very last chunk, by two PE ones-vector matmuls
                    # into PSUM (no DMA latency on the critical tail)
                    dsc = sb.tile([P, 512], F32, tag="dsc", bufs=2)
                    nc.vector.tensor_copy(dsc[0:1, :], av_e[HD:65, :])
                    nc.vector.tensor_copy(dsc[HD:65, :], av_o[HD:65, :])
                    rr = sb.tile([P, 512], F32, tag="rr", bufs=2)
                    nc.vector.reciprocal_approx_fast(rr[0:65, :], dsc[0:65, :])
                    rr16 = sb.tile([P, 512], F16, tag="rr16", bufs=2)
                    nc.vector.tensor_copy(rr16[0:65, :], rr[0:65, :])
                    nc.sync.dma_start(rscr[m, j, 0], rr16[0:1, :])
                    nc.sync.dma_start(rscr[m, j, 1], rr16[HD:65, :])
                    rb = sb.tile([P, 512], F16, tag="rb", bufs=4)
                    for h in range(2):
                        nc.sync.dma_start(
                            rb[h * HD:(h + 1) * HD, :],
                            rscr[m, j, h][None, :].to_broadcast((HD, 512)))
                    nc.vector.tensor_tensor(opj[m][j][:], opj[m][j][:],
                                            rb[:], mybir.AluOpType.mult)

            # ---------- phase 3: output projection ----------
            # Runs inside the attention pools' scope, reusing the score
            # ring's PSUM tiles (two column chunks per [128,1024] tile): no
            # pool-transition barrier, so the PE rolls straight from the
            # last AV chain into the output projection while the last query
            # chunk still normalizes. Casts alternate Vector/Scalar; 4
            # grouped output DMAs go on the Scalar queue.
            osb = sb.tile([P, S // P, D], F16)
            for sg in range(S // P // 2):
                ps = apool.tile([P, 1024], F32, tag="sc")
                for half in range(2):
                    si = 2 * sg + half
                    sl = slice((si % 4) * P, (si % 4 + 1) * P)
                    po = ps[:, half * 512:(half + 1) * 512]
                    for m in range(2):
                        lhs = opj[m][si // 4][:, sl]
                        nc.tensor.matmul(po, lhs, wot[:, m, :],
                                         start=(m == 0), stop=(m == 1))
                    # vector is busy with the last chunk's 1/den chain at the
                    # start of this phase, so lean on scalar first
                    eng = nc.scalar if si < 6 or si % 2 == 1 else nc.vector
                    cast(eng, osb[:, si, :], po)
                if sg % 2 == 1:
                    g = sg // 2
                    nc.scalar.dma_start(
                        out.rearrange("(a p) d -> p a d", p=P)[:, g * 4:(g + 1) * 4, :],
                        osb[:, g * 4:(g + 1) * 4, :])

    nc.compile()
    return nc


def kernel(x, mask, Wq, bq, Wk, bk, Wv, bv, Wo, bo):
    x = np.asarray(x, np.float32)
    mask = np.asarray(mask)
    Wq, bq = np.asarray(Wq, np.float32), np.asarray(bq, np.float32)
    Wk, bk = np.asarray(Wk, np.float32), np.asarray(bk, np.float32)
    Wv, bv = np.asarray(Wv, np.float32), np.asarray(bv, np.float32)
    Wo, bo = np.asarray(Wo, np.float32), np.asarray(bo, np.float32)

    aug = any(np.any(bias != 0) for bias in (bq, bk, bv))
    kt = 5 if aug else 4
    da = kt * P

    idxs = [np.nonzero(mask[b])[0] for b in range(B)]
    nsk = max(1, max((len(ix) + P - 1) // P for ix in idxs))
    SK = nsk * P

    in_maps = []
    for c in range(8):
        b, half = c // 2, c % 2
        ix = idxs[b]
        n = len(ix)
        hs = slice(half * HPC * HD, (half + 1) * HPC * HD)   # 256 head columns

        xTb = x[b].T.astype(np.float16)
        xTa = np.zeros((da, S), np.float16)
        xTa[:D] = xTb
        xkT = np.zeros((da, SK), np.float16)
        xkT[:D, :n] = xTb[:, ix]

        wk_a = np.zeros((da, HPC * HD), np.float16)
        wk_a[:D] = Wk[:, hs].astype(np.float16)
        wq_a = np.zeros((da, HPC * HD), np.float16)
        wq_a[:D] = Wq[:, hs].astype(np.float16)
        wv_a = np.zeros((da, HPC * 65), np.float16)
        for l in range(HPC):
            hg = half * HPC + l
            wv_a[:D, l * 65:l * 65 + HD] = Wv[:, hg * HD:(hg + 1) * HD].astype(np.float16)

        masktf = (np.arange(SK) < n).astype(np.float32).reshape(nsk, P).T.copy()

        if aug:
            xTa[D] = 1.0
            xkT[D, :n] = 1.0
            wk_a[D] = bk[hs].astype(np.float16)
            wq_a[D] = bq[hs].astype(np.float16)
            for l in range(HPC):
                hg = half * HPC + l
                wv_a[D, l * 65:l * 65 + HD] = bv[hg * HD:(hg + 1) * HD].astype(np.float16)
                wv_a[D, l * 65 + HD] = 1.0

        wo_a = np.stack(
            [Wo[(half * HPC + 2 * m) * HD:(half * HPC + 2 * m + 2) * HD, :]
             for m in range(2)]
        ).astype(np.float16)

        in_maps.append({
            "xT": xTa, "xkT": xkT, "wk": wk_a, "wq": wq_a, "wv": wv_a,
            "wo": wo_a, "maskt": masktf,
        })

    nc = _build(aug, nsk)
    import os
    trace = bool(int(os.environ.get("MHA_TRACE", "0")))
    res = bass_utils.run_bass_kernel_spmd(nc, in_maps, core_ids=list(range(8)),
                                          trace=trace)
    global last_result
    last_result = res

    outf = np.empty((B, S, D), np.float32)
    for b in range(B):
        outf[b] = (res.results[2 * b]["out"].astype(np.float32)
                   + res.results[2 * b + 1]["out"].astype(np.float32)
                   + bo[None, :])
    return outf


# revision 42
# speedup vs baseline: 1.0232x; 1.0232x over previous
"""Multi-head attention (B=4, S=2048, D=512, H=8) on 8 Trainium2 NeuronCores.

Sharding: core c handles batch b = c//2 and heads [4*(c%2) .. 4*(c%2)+3]
(data parallel on B, tensor parallel on H). Each core computes Q/K/V
projections for its 4 heads, per-head attention, and a partial output
projection (its 256 rows of Wo). The host sums the two partial outputs per
batch and adds bo.

Perf structure (fp32 baseline 522us -> fp16 151us -> this, ~133us):
 - All matmul operands fp16: 1 PE cycle/column (fp32 is 4). ~1e-3 rel err.
 - Masked keys are compacted away on the host (padded to a multiple of 128);
   padding keys produce score 0 -> exp 1, but their V' rows and ones-column
   are 0 so they contribute nothing to AV numerator or softmax denominator.
 - The Scalar engine's exp is the attention pacer (~1.1us per [128,1024]
   instruction covering both heads of a pair, 72 instructions, back-to-back
   for ~87us). Everything else is arranged to hide under or around it:
     * all input DMAs share one queue in priority order (K inputs first, Q
       chunks last) — the 16 underlying DMA engines are bandwidth-shared
       (~350GB/s aggregate), so competing queues would starve the first
       projections; junk matmuls warm the PE clock (0.65->2.4GHz) and a
       dummy exp preloads the activation table while inputs stream;
     * only K0/V/Q0-chunk0 projections run up front; remaining Q0 chunks
       and pair-1's K/Q chains are interleaved into pair-0's attention,
       borrowing score-ring PSUM banks in the PE's idle gaps;
     * softmax denominators are folded into V' (65th row); each query chunk
       is normalized eagerly right after its AV chain using
       reciprocal_approx_fast (single DVE op) and one DRAM round trip to
       broadcast 1/den across partitions (the odd head's O rows bounce
       through DRAM anyway to reach partitions 64..127);
     * the output projection reuses the score ring's PSUM tiles inside the
       attention pools' scope (a fresh pool would pay a 15-20us transition
       barrier); PSUM evacuation casts alternate Vector/Scalar (the only
       PSUM-capable engines) and 4 grouped DMAs ship the fp16 result.
 - Empirical pitfalls baked into the structure: SBUF layout is
   perf-sensitive (shifting the exp output tile by 128B cost 220ns on every
   exp — keep late-added tiles allocated after the hot ones); mixing
   K=128 and K=64 matmuls in one PSUM accumulation group hard-faults the
   device; GPSIMD and DMA cannot touch PSUM.
"""

import numpy as np
from contextlib import ExitStack

import concourse.bass as bass
from concourse.bacc import Bacc
import concourse.mybir as mybir
import concourse.tile as tile
from concourse import bass_utils

F32 = mybir.dt.float32
F16 = mybir.dt.float16
B, S, D, H, HD = 4, 2048, 512, 8, 64
P = 128
HPC = 4            # heads per core
NSQ = S // 512     # 4 query chunks of 512


def _build(aug: bool, nsk: int) -> bass.Bass:
    kt = 5 if aug else 4
    da = kt * P
    SK = nsk * P
    nc = Bacc(trn_type="TRN2")

    xT = nc.dram_tensor("xT", [da, S], F16, kind="ExternalInput")
    xkT = nc.dram_tensor("xkT", [da, SK], F16, kind="ExternalInput")
    wk = nc.dram_tensor("wk", [da, HPC * HD], F16, kind="ExternalInput")
    wv = nc.dram_tensor("wv", [da, HPC * 65], F16, kind="ExternalInput")
    wq = nc.dram_tensor("wq", [da, HPC * HD], F16, kind="ExternalInput")
    wo = nc.dram_tensor("wo", [2, P, D], F16, kind="ExternalInput")
    maskt = nc.dram_tensor("maskt", [P, nsk], F32, kind="ExternalInput")
    out = nc.dram_tensor("out", [S, D], F16, kind="ExternalOutput")

    with tile.TileContext(nc) as tc, ExitStack() as ctx:
        sb = ctx.enter_context(tc.tile_pool(name="sb", bufs=1))
        dram = ctx.enter_context(tc.tile_pool(name="dram", bufs=1, space="DRAM"))

        # ---------- input DMAs: one queue, priority order ----------
        xTr = xT.rearrange("(t p) m -> p t m", p=P)
        wkt = sb.tile([P, kt, HPC * HD], F16)
        nc.sync.dma_start(wkt[:], wk.rearrange("(t p) m -> p t m", p=P))
        xkt = sb.tile([P, kt, SK], F16)
        xkr = xkT.rearrange("(t p) m -> p t m", p=P)
        for (lo, hi) in [(c, min(c + 512, SK)) for c in range(0, SK, 512)]:
            nc.sync.dma_start(xkt[:, :, lo:hi], xkr[:, :, lo:hi])
        mkt = sb.tile([P, nsk], F32)
        nc.sync.dma_start(mkt[:], maskt[:])
        wvt = sb.tile([P, kt, HPC * 65], F16)
        nc.sync.dma_start(wvt[:], wv.rearrange("(t p) m -> p t m", p=P))
        wqt = sb.tile([P, kt, HPC * HD], F16)
        nc.sync.dma_start(wqt[:], wq.rearrange("(t p) m -> p t m", p=P))
        xt = sb.tile([P, kt, S], F16)
        for j in range(NSQ):
            nc.sync.dma_start(xt[:, :, j * 512:(j + 1) * 512],
                              xTr[:, :, j * 512:(j + 1) * 512])
        wot = sb.tile([P, 2, D], F16)
        nc.sync.dma_start(wot[:], wo.rearrange("m p d -> p m d"))

        ktile = [sb.tile([P, SK], F16, tag=f"kT{m}", name=f"kT{m}") for m in range(2)]
        qtile = [sb.tile([P, S], F16, tag=f"qT{m}", name=f"qT{m}") for m in range(2)]
        vt = sb.tile([P, nsk, HPC * 65], F16)
        # one O' tile per (pair, query chunk): the output projection's
        # dependency tracking is per-tile, so chunk-sized tiles let early
        # column chunks project while the last chunk still normalizes
        opj = [[sb.tile([P, 512], F16, tag=f"op{m}_{j}", name=f"op{m}_{j}")
                for j in range(NSQ)] for m in range(2)]
        oscr = dram.tile([2, NSQ, HD, 512], F16)           # odd-head O bounce
        rscr = dram.tile([2, NSQ, 2, 512], F16)            # 1/den bounce
        kchunks = [(c, min(c + 512, SK)) for c in range(0, SK, 512)]

        def cast(eng, dst, src):
            if eng is nc.scalar:
                nc.scalar.copy(dst, src)
            else:
                eng.tensor_copy(dst, src)

        def proj_k(m, pool, lo, hi, eng=nc.vector):
            ps = pool.tile([P, 1024], F32, tag="sc")
            for t in range(kt):
                nc.tensor.matmul(
                    ps[:, 0:hi - lo], wkt[:, t, m * P:(m + 1) * P],
                    xkt[:, t, lo:hi], start=(t == 0), stop=(t == kt - 1))
            cast(eng, ktile[m][:, lo:hi], ps[:, 0:hi - lo])

        def proj_v(pool, si, eng=nc.vector):
            # natural orientation [keys, 4*65], all four heads at once, two
            # key chunks per PSUM tile; the ones-columns then get 1*mask
            # (key-validity) via tiny gpsimd ops
            n2 = min(2, nsk - si)
            ps = pool.tile([P, 1024], F32, tag="sc")
            for h in range(n2):
                for t in range(kt):
                    nc.tensor.matmul(
                        ps[:, h * 512:h * 512 + HPC * 65],
                        xkt[:, t, (si + h) * P:(si + h + 1) * P],
                        wvt[:, t, :], start=(t == 0), stop=(t == kt - 1))
            engs2 = [eng, nc.scalar if eng is nc.vector else nc.vector]
            for h in range(n2):
                cast(engs2[h], vt[:, si + h, :], ps[:, h * 512:h * 512 + HPC * 65])
            if not aug:
                for h in range(n2):
                    ones = vt[:, si + h, HD::65]
                    nc.gpsimd.memset(ones, 1.0)
                    nc.gpsimd.tensor_scalar_mul(ones, ones, mkt[:, si + h:si + h + 1])

        def proj_q(m, pool, j, eng=nc.vector):
            ps = pool.tile([P, 1024], F32, tag="sc")
            for t in range(kt):
                nc.tensor.matmul(
                    ps[:, 0:512], wqt[:, t, m * P:(m + 1) * P],
                    xt[:, t, j * 512:(j + 1) * 512],
                    start=(t == 0), stop=(t == kt - 1))
            cast(eng, qtile[m][:, j * 512:(j + 1) * 512], ps[:, 0:512])

        # ---------- phase 1: minimal up-front projections ----------
        with tc.tile_pool(name="proj_ps", bufs=2, space="PSUM") as ppool:
            # Junk matmul chains while the input DMAs stream in: keeps the
            # PE busy from the preamble on so its clock is fully ramped
            # (0.65 -> 2.4 GHz takes a few us of activity) when the real
            # projections start. Results are never read.
            junk = sb.tile([P, 512], F16, tag="junk")
            nc.vector.memset(junk[:], 0.0)
            # preload the Exp activation table while DMAs stream
            jp = sb.tile([P, 1], F16, tag="jp")
            nc.scalar.activation(jp[:], junk[:, 0:1],
                                 mybir.ActivationFunctionType.Exp)
            for _ in range(5):
                ps = ppool.tile([P, 1024], F32, tag="sc")
                for r in range(4):
                    nc.tensor.matmul(ps[:, 0:512], junk[:, 0:P], junk[:],
                                     start=(r == 0), stop=(r == 3))
            engs = [nc.vector, nc.scalar]
            n = 0
            for (lo, hi) in kchunks:
                proj_k(0, ppool, lo, hi, engs[n % 2]); n += 1
            for si in range(0, nsk, 2):
                proj_v(ppool, si, engs[n % 2]); n += 1
            proj_q(0, ppool, 0, engs[n % 2]); n += 1

        # chains to interleave into pair-0's attention (PE idle gaps)
        pending = []
        for (lo, hi) in kchunks:
            pending.append(lambda pool, lo=lo, hi=hi: proj_k(1, pool, lo, hi))
        for j in range(NSQ):
            pending.append(lambda pool, j=j: proj_q(1, pool, j))

        # ---------- phase 2: attention ----------
        with tc.tile_pool(name="attn_ps", bufs=2, space="PSUM") as apool, \
             tc.tile_pool(name="av_ps", bufs=2, space="PSUM") as avpool:
            for m in range(2):                              # head pair
                le, lo_ = 2 * m, 2 * m + 1
                if m == 1:
                    while pending:                          # safety net
                        pending.pop(0)(apool)
                for j in range(NSQ):                        # query chunk of 512
                    qe = qtile[m][0:HD, j * 512:(j + 1) * 512]
                    qo = qtile[m][HD:P, j * 512:(j + 1) * 512]
                    av_e = avpool.tile([65, 512], F32, tag="av_e")
                    av_o = avpool.tile([65, 512], F32, tag="av_o")

                    def emit_av(sk, p):
                        st = dict(start=(sk == 0), stop=(sk == nsk - 1))
                        nc.tensor.matmul(av_e[:], vt[:, sk, le * 65:le * 65 + 65],
                                         p[:, 0:512], **st)
                        nc.tensor.matmul(av_o[:], vt[:, sk, lo_ * 65:lo_ * 65 + 65],
                                         p[:, 512:1024], **st)

                    prev = None
                    for sk in range(nsk):
                        sc = apool.tile([P, 1024], F32, tag="sc")
                        nc.tensor.matmul(
                            sc[:, 0:512], ktile[m][0:HD, sk * P:(sk + 1) * P], qe,
                            start=True, stop=True)
                        nc.tensor.matmul(
                            sc[:, 512:1024], ktile[m][HD:P, sk * P:(sk + 1) * P], qo,
                            start=True, stop=True)
                        p = sb.tile([P, 1024], F16, tag="p", bufs=4)
                        nc.scalar.activation(p[:], sc[:],
                                             mybir.ActivationFunctionType.Exp,
                                             scale=0.125)
                        if prev is not None:
                            emit_av(*prev)
                        prev = (sk, p)
                        if m == 0:
                            # next Q0 chunk first, then pair-1 chains; late
                            # slots so the previous chunk's DVE burst (den
                            # chain + O copies) has drained and the inserted
                            # cast isn't queued behind it
                            if sk == 2 and j < NSQ - 1:
                                proj_q(0, apool, j + 1)
                            elif sk in (4, 6) and pending:
                                pending.pop(0)(apool)
                    emit_av(*prev)

                    # O rows out first so the odd head's DRAM bounce is in
                    # flight while the denominator chain runs
                    osh = sb.tile([HD, 512], F16, tag="osh", bufs=2)
                    nc.vector.tensor_copy(osh[:], av_o[0:HD, :])
                    nc.gpsimd.dma_start(oscr[m, j], osh[:])
                    nc.gpsimd.dma_start(opj[m][j][HD:P, :], oscr[m, j])
                    nc.vector.tensor_copy(opj[m][j][0:HD, :], av_e[0:HD, :])
                    # eager normalization: den rows -> 1/den (fast approx),
                    # broadcast across partitions by a DRAM round trip — or,
                    # for the very last chunk, by two PE ones-vector matmuls
                    # into PSUM (no DMA latency on the critical tail)
                    dsc = sb.tile([P, 512], F32, tag="dsc", bufs=2)
                    nc.vector.tensor_copy(dsc[0:1, :], av_e[HD:65, :])
                    nc.vector.tensor_copy(dsc[HD:65, :], av_o[HD:65, :])
                    rr = sb.tile([P, 512], F32, tag="rr", bufs=2)
                    nc.vector.reciprocal_approx_fast(rr[0:65, :], dsc[0:65, :])
                    rr16 = sb.tile([P, 512], F16, tag="rr16", bufs=2)
                    nc.vector.tensor_copy(rr16[0:65, :], rr[0:65, :])
                    nc.sync.dma_start(rscr[m, j, 0], rr16[0:1, :])
                    nc.sync.dma_start(rscr[m, j, 1], rr16[HD:65, :])
                    rb = sb.tile([P, 512], F16, tag="rb", bufs=4)
                    for h in range(2):
                        nc.sync.dma_start(
                            rb[h * HD:(h + 1) * HD, :],
                            rscr[m, j, h][None, :].to_broadcast((HD, 512)))
                    nc.vector.tensor_tensor(opj[m][j][:], opj[m][j][:],
                                            rb[:], mybir.AluOpType.mult)

            # ---------- phase 3: output projection ----------
            # Runs inside the attention pools' scope, reusing the score
            # ring's PSUM tiles (two column chunks per [128,1024] tile): no
            # pool-transition barrier, so the PE rolls straight from the
            # last AV chain into the output projection while the last query
            # chunk still normalizes. Casts alternate Vector/Scalar; 4
            # grouped output DMAs go on the Scalar queue.
            osb = sb.tile([P, S // P, D], F16)
            for sg in range(S // P // 2):
                ps = apool.tile([P, 1024], F32, tag="sc")
                for half in range(2):
                    si = 2 * sg + half
                    sl = slice((si % 4) * P, (si % 4 + 1) * P)
                    po = ps[:, half * 512:(half + 1) * 512]
                    for m in range(2):
                        lhs = opj[m][si // 4][:, sl]
                        nc.tensor.matmul(po, lhs, wot[:, m, :],
                                         start=(m == 0), stop=(m == 1))
                    # vector is busy with the last chunk's 1/den chain at the
                    # start of this phase, so lean on scalar first
                    eng = nc.scalar if si < 4 or si % 2 == 1 else nc.vector
                    cast(eng, osb[:, si, :], po)
                # 8 half-size output DMAs on alternating queues: smaller
                # final transfer and neither queue serializes the tail
                dmae = nc.sync if sg % 2 == 0 else nc.scalar
                dmae.dma_start(
                    out.rearrange("(a p) d -> p a d", p=P)[:, sg * 2:(sg + 1) * 2, :],
                    osb[:, sg * 2:(sg + 1) * 2, :])

    nc.compile()
    return nc


def kernel(x, mask, Wq, bq, Wk, bk, Wv, bv, Wo, bo):
    x = np.asarray(x, np.float32)
    mask = np.asarray(mask)
    Wq, bq = np.asarray(Wq, np.float32), np.asarray(bq, np.float32)
    Wk, bk = np.asarray(Wk, np.float32), np.asarray(bk, np.float32)
    Wv, bv = np.asarray(Wv, np.float32), np.asarray(bv, np.float32)
    Wo, bo = np.asarray(Wo, np.float32), np.asarray(bo, np.float32)

    aug = any(np.any(bias != 0) for bias in (bq, bk, bv))
    kt = 5 if aug else 4
    da = kt * P

    idxs = [np.nonzero(mask[b])[0] for b in range(B)]
    nsk = max(1, max((len(ix) + P - 1) // P for ix in idxs))
    SK = nsk * P

    in_maps = []
    for c in range(8):
        b, half = c // 2, c % 2
        ix = idxs[b]
        n = len(ix)
        hs = slice(half * HPC * HD, (half + 1) * HPC * HD)   # 256 head columns

        xTb = x[b].T.astype(np.float16)
        xTa = np.zeros((da, S), np.float16)
        xTa[:D] = xTb
        xkT = np.zeros((da, SK), np.float16)
        xkT[:D, :n] = xTb[:, ix]

        wk_a = np.zeros((da, HPC * HD), np.float16)
        wk_a[:D] = Wk[:, hs].astype(np.float16)
        wq_a = np.zeros((da, HPC * HD), np.float16)
        wq_a[:D] = Wq[:, hs].astype(np.float16)
        wv_a = np.zeros((da, HPC * 65), np.float16)
        for l in range(HPC):
            hg = half * HPC + l
            wv_a[:D, l * 65:l * 65 + HD] = Wv[:, hg * HD:(hg + 1) * HD].astype(np.float16)

        masktf = (np.arange(SK) < n).astype(np.float32).reshape(nsk, P).T.copy()

        if aug:
            xTa[D] = 1.0
            xkT[D, :n] = 1.0
            wk_a[D] = bk[hs].astype(np.float16)
            wq_a[D] = bq[hs].astype(np.float16)
            for l in range(HPC):
                hg = half * HPC + l
                wv_a[D, l * 65:l * 65 + HD] = bv[hg * HD:(hg + 1) * HD].astype(np.float16)
                wv_a[D, l * 65 + HD] = 1.0

        wo_a = np.stack(
            [Wo[(half * HPC + 2 * m) * HD:(half * HPC + 2 * m + 2) * HD, :]
             for m in range(2)]
        ).astype(np.float16)

        in_maps.append({
            "xT": xTa, "xkT": xkT, "wk": wk_a, "wq": wq_a, "wv": wv_a,
            "wo": wo_a, "maskt": masktf,
        })

    nc = _build(aug, nsk)
    import os
    trace = bool(int(os.environ.get("MHA_TRACE", "0")))
    res = bass_utils.run_bass_kernel_spmd(nc, in_maps, core_ids=list(range(8)),
                                          trace=trace)
    global last_result
    last_result = res

    outf = np.empty((B, S, D), np.float32)
    for b in range(B):
        outf[b] = (res.results[2 * b]["out"].astype(np.float32)
                   + res.results[2 * b + 1]["out"].astype(np.float32)
                   + bo[None, :])
    return outf
